# revision 17
# baseline (speedup 1.0000x reference)
"""Trainium2 Bass kernel for nn_Atten2Map (DeePMD dpa2 Atten2Map-style sparse attention).

Contract: kernel(**inputs) takes FULL unsharded numpy inputs
(g2 [2,512,128,64], h2 [2,512,128,3], nlist_mask [2,512,128] bool,
sw [2,512,128], Wqk [64,512]) and returns the full output
[2,512,128,128,4] float32. Internally shards the nb*nloc=1024 atoms
data-parallel across 8 NeuronCores.

Math per atom (nnei=128 neighbors, ND=64, NH=4 heads):
  qk   = g2 @ Wqk                  -> q_h, k_h     [128, 64] each
  raw  = q_h @ k_h^T / sqrt(64)    (scores)
  hh   = h2 @ h2^T                 (gate)
  t    = (raw * hh + 20) * sw_i * sw_j - 20
  a    = softmax(t, axis=-1)  (constant shifts cancel in softmax)
  out[i, j, h] = a * mask_i * mask_j * sw_i * sw_j * hh / sqrt(3)

Device formulation (v2 — engine-balanced):
  host: g2s = g2 * sw            (so scores come out of PE already * sw_i*sw_j)
        W2_h = Wq_h @ Wk_h^T / 8 (64x64 per head; scores = G' W2 G'^T)
  PE:   ptm  = w2p2_hp^T @ G'^T           2-head-merged tmp  [128, 256] x2
        X'_h = tmpT_h^T @ G'^T            scores*sw_i*sw_j   [128, 128] x4/atom
        hh|hm = ht^T @ [ht|htm]           gates              [128, 256]/atom
  GPSIMD: v1 = X' (*) hh                  (PSUM read, gate multiply)
  ACT:  E1 = exp(v1 - 60)  -> bf16
        F  = exp(20*sw_i * sw_j) -> bf16  (scale=per-partition 20*sw_i)
  DVE:  E = E1 (*) F (bf16, 4x mode);  rows = reduce_X(E) f32
        rinva = (1/rows) * mask_i*sw_i
        ti_h = (E_h * rinva_h) * hm       (bf16 4x, per-head planes)
  out DRAM [A, 128(i), 4(h), 128(j)] bf16; host -> f32 transpose/reshape.
"""

import numpy as np
from contextlib import ExitStack

import ml_dtypes
import concourse.bass as bass
import concourse.tile as tile
from concourse import bacc, mybir
from concourse.bass_utils import run_bass_kernel_spmd

ND, NH, SHIFT = 64, 4, 20.0
NNEI, DIN = 128, 64
NCORES = 8
EXPB = 60.0  # constant shift inside exp; cancels in softmax normalization

F32 = mybir.dt.float32
F16 = mybir.dt.float16
BF16 = mybir.dt.bfloat16
DTE = mybir.dt.bfloat16  # post-exp path dtype

P = NNEI     # 128
BLK = 16     # atoms per DMA block
GRP = 4      # atoms per output DMA group


def build_nc(A: int):
    """Build the per-core Bass program for A atoms (A multiple of BLK)."""
    assert A % BLK == 0
    nc = bacc.Bacc("TRN2", target_bir_lowering=False, debug=False, num_devices=NCORES)
    dp = nc.declare_dram_parameter
    g2sT = dp("g2sT", [A, DIN, P], F16, isOutput=False)    # sw-folded g2, transposed
    hcat = dp("hcat", [A, 6, P], F16, isOutput=False)      # rows 0:3 h2T, 3:6 (h2*mask*sw/sqrt3)T
    w2p = dp("w2p", [DIN, NH * ND], F16, isOutput=False)   # per-head W2, head-major cols
    sws = dp("sws", [P, 2 * A], F32, isOutput=False)       # [20*swiT | rmT]
    swrow = dp("swrow", [1, A * P], F32, isOutput=False)   # flat sw rows for broadcast
    out = dp("out", [A, P, NH * P], DTE, isOutput=True)   # [atom, i, (h, j)]

    AF = mybir.ActivationFunctionType
    OP = mybir.AluOpType

    with tile.TileContext(nc) as tc, ExitStack() as ctx:
        sb = ctx.enter_context(tc.tile_pool(name="persist", bufs=1))
        w2p_s = sb.tile([DIN, NH * ND], F16)
        nc.sync.dma_start(w2p_s[:, :], w2p[:, :])
        sws_s = sb.tile([P, 2 * A], F32)
        nc.sync.dma_start(sws_s[:, :], sws[:, :])
        swi20T_s = sws_s[:, 0:A]
        rmT_s = sws_s[:, A:2 * A]
        negb = sb.tile([P, 1], F32)
        nc.vector.memset(negb[:, :], -EXPB)

        # block-level input pools
        gt_pool = ctx.enter_context(tc.tile_pool(name="gt", bufs=2))
        hc_pool = ctx.enter_context(tc.tile_pool(name="hc", bufs=2))
        swj_pool = ctx.enter_context(tc.tile_pool(name="swj", bufs=2))
        # pair/atom-level pools
        tts_pool = ctx.enter_context(tc.tile_pool(name="tts", bufs=3))
        hh_pool = ctx.enter_context(tc.tile_pool(name="hh", bufs=3))
        v1_pool = ctx.enter_context(tc.tile_pool(name="v1", bufs=3))
        w20_pool = ctx.enter_context(tc.tile_pool(name="w20", bufs=3))
        v2_pool = ctx.enter_context(tc.tile_pool(name="v2", bufs=3))
        e_pool = ctx.enter_context(tc.tile_pool(name="e", bufs=3))
        e2_pool = ctx.enter_context(tc.tile_pool(name="e2", bufs=3))
        stat_pool = ctx.enter_context(tc.tile_pool(name="stat", bufs=4))
        stage_pool = ctx.enter_context(tc.tile_pool(name="stage", bufs=2))
        # PSUM pools
        ptm_pool = ctx.enter_context(tc.tile_pool(name="ptm", bufs=2, space="PSUM"))
        psc_pool = ctx.enter_context(tc.tile_pool(name="psc", bufs=2, space="PSUM"))
        phh_pool = ctx.enter_context(tc.tile_pool(name="phh", bufs=2, space="PSUM"))

        for blk in range(A // BLK):
            b0 = blk * BLK
            gts = gt_pool.tile([DIN, BLK * P], F16)
            nc.sync.dma_start(gts[:, :].rearrange("p (a j) -> p a j", a=BLK),
                              g2sT[b0:b0 + BLK, :, :].rearrange("a p j -> p a j"))
            hcs = hc_pool.tile([3, BLK * P], F16, tag="ht")
            nc.sync.dma_start(hcs[:, :].rearrange("p (a j) -> p a j", a=BLK),
                              hcat[b0:b0 + BLK, 0:3, :].rearrange("a p j -> p a j"))
            hms = hc_pool.tile([3, BLK * P], F16, tag="htm")
            nc.sync.dma_start(hms[:, :].rearrange("p (a j) -> p a j", a=BLK),
                              hcat[b0:b0 + BLK, 3:6, :].rearrange("a p j -> p a j"))
            swjb = swj_pool.tile([P, BLK * P], F32)
            nc.sync.dma_start(swjb[:, :],
                              swrow[0:1, b0 * P:(b0 + BLK) * P].broadcast_to([P, BLK * P]))

            stage = None
            for lp in range(BLK // 2):
                la0 = 2 * lp          # block-local atom indices
                a0 = b0 + la0         # global (per-core) atom index
                gpair = gts[:, la0 * P:(la0 + 2) * P]          # [64, 256]

                # --- tmp matmuls: 2-head-merged, per pair ---
                ptm = ptm_pool.tile([P, 2 * 2 * P], F32)
                for hp in range(2):
                    nc.tensor.matmul(ptm[:, hp * 2 * P:(hp + 1) * 2 * P],
                                     w2p_s[:, hp * 2 * ND:(hp + 1) * 2 * ND],
                                     gpair, start=True, stop=True)
                tts = tts_pool.tile([P, 2 * 2 * P], F16, tag="tts")
                nc.scalar.copy(tts[:, :], ptm[:, :])
                tts2 = tts_pool.tile([ND, 2 * 2 * P], F16, tag="tts2")
                nc.sync.dma_start(tts2[:, :], tts[ND:, :])

                # --- gate matmuls: hh|hm per atom, [128, 512] per pair ---
                phh = phh_pool.tile([P, 4 * P], F32)
                for ai in range(2):
                    hta = hcs[:, (la0 + ai) * P:(la0 + ai + 1) * P]
                    htma = hms[:, (la0 + ai) * P:(la0 + ai + 1) * P]
                    nc.tensor.matmul(phh[:, (2 * ai) * P:(2 * ai + 1) * P],
                                     hta, hta, start=True, stop=True)
                    nc.tensor.matmul(phh[:, (2 * ai + 1) * P:(2 * ai + 2) * P],
                                     hta, htma, start=True, stop=True)
                hhm = hh_pool.tile([P, 4 * P], F16, tag="hhm")
                nc.scalar.copy(hhm[:, :], phh[:, :])

                # --- score matmuls: per atom per head, N=128 ---
                psc = psc_pool.tile([P, 2 * NH * P], F32)
                for ai in range(2):
                    for hp in range(2):
                        for hi in range(2):
                            h = 2 * hp + hi
                            src_t = tts if hi == 0 else tts2
                            lhsT = src_t[0:ND,
                                         hp * 2 * P + ai * P:hp * 2 * P + (ai + 1) * P]
                            ga = gts[:, (la0 + ai) * P:(la0 + ai + 1) * P]
                            nc.tensor.matmul(psc[:, (ai * NH + h) * P:(ai * NH + h + 1) * P],
                                             lhsT, ga, start=True, stop=True)

                # --- v1 = X' * hh   (DVE, PSUM read; gpsimd cannot touch PSUM) ---
                v1 = v1_pool.tile([P, 2 * NH * P], F32, tag="v1")
                hh_b = hhm[:, :].rearrange("p (a k j) -> p a k j", a=2, k=2)[:, :, 0:1, :] \
                    .broadcast_to([P, 2, NH, P])
                nc.vector.tensor_tensor(
                    v1[:, :].rearrange("p (a h j) -> p a h j", a=2, h=NH),
                    psc[:, :].rearrange("p (a h j) -> p a h j", a=2, h=NH),
                    hh_b, op=OP.mult)

                # --- w20 = 20*swi * swj (scalar ACT copy, exact f32) ---
                w20 = w20_pool.tile([P, 2 * P], F32, tag="w20")
                for ai in range(2):
                    nc.vector.tensor_scalar(
                        w20[:, ai * P:(ai + 1) * P],
                        swjb[:, (la0 + ai) * P:(la0 + ai + 1) * P],
                        swi20T_s[:, a0 + ai:a0 + ai + 1], None, op0=OP.mult)

                # --- v2 = v1 + w20 (gpsimd, all-SBUF) ---
                v2 = v2_pool.tile([P, 2 * NH * P], F32, tag="v2")
                w20_b = w20[:, :].rearrange("p (a j) -> p a j", a=2) \
                    .unsqueeze(2).broadcast_to([P, 2, NH, P])
                nc.gpsimd.tensor_tensor(
                    v2[:, :].rearrange("p (a h j) -> p a h j", a=2, h=NH),
                    v1[:, :].rearrange("p (a h j) -> p a h j", a=2, h=NH),
                    w20_b, op=OP.add)

                # --- E = exp(v2 - 60) -> bf16 (pair-wide) ---
                e_t = e_pool.tile([P, 2 * NH * P], DTE, tag="e")
                nc.scalar.activation(e_t[:, :], v2[:, :], AF.Exp,
                                     bias=negb[:, 0:1], scale=1.0)

                # --- rows = sum_j E; rinva = mask_i*swi / rows (bf16) ---
                rows = stat_pool.tile([P, 4 * NH], F32, tag="rows")
                nc.vector.tensor_reduce(
                    rows[:, 0:2 * NH],
                    e_t[:, :].rearrange("p (k j) -> p k j", k=2 * NH),
                    axis=mybir.AxisListType.X, op=OP.add)
                nc.vector.reciprocal(rows[:, 2 * NH:4 * NH], rows[:, 0:2 * NH])
                rinva = stat_pool.tile([P, 2 * NH], DTE, tag="rinva")
                for ai in range(2):
                    nc.vector.tensor_scalar(
                        rinva[:, ai * NH:(ai + 1) * NH],
                        rows[:, 2 * NH + ai * NH:2 * NH + (ai + 1) * NH],
                        rmT_s[:, a0 + ai:a0 + ai + 1], None, op0=OP.mult)

                # --- out: ti = E * rinva * hm as two pair-wide bf16 4x TTs ---
                gi = (la0 // GRP)
                if la0 % GRP == 0:
                    stage = stage_pool.tile([P, GRP * NH * P], DTE, tag="stage")
                soff = (la0 % GRP) * NH * P
                e2 = e2_pool.tile([P, 2 * NH * P], DTE, tag="e2")
                rinva_b = rinva[:, :].rearrange("p (a h) -> p a h", a=2) \
                    .unsqueeze(3).broadcast_to([P, 2, NH, P])
                nc.vector.tensor_tensor(
                    e2[:, :].rearrange("p (a h j) -> p a h j", a=2, h=NH),
                    e_t[:, :].rearrange("p (a h j) -> p a h j", a=2, h=NH),
                    rinva_b, op=OP.mult)
                hm_b = hhm[:, :].rearrange("p (a k j) -> p a k j", a=2, k=2)[:, :, 1:2, :] \
                    .broadcast_to([P, 2, NH, P])
                nc.gpsimd.tensor_tensor(
                    stage[:, soff:soff + 2 * NH * P]
                        .rearrange("p (a h j) -> p a h j", a=2, h=NH),
                    e2[:, :].rearrange("p (a h j) -> p a h j", a=2, h=NH),
                    hm_b, op=OP.mult)
                if la0 % GRP == GRP - 2:
                    g0 = b0 + gi * GRP
                    nc.sync.dma_start(
                        out[g0:g0 + GRP, :, :].rearrange("a p q -> p a q"),
                        stage[:, :].rearrange("p (a q) -> p a q", a=GRP))
    if not nc.is_finalized():
        nc.finalize()
    return nc


def _host_prep(g2, h2, nlist_mask, sw, Wqk):
    """Build per-core input maps (host-side numpy prep)."""
    nb, nloc, nnei, din = g2.shape
    ATOT = nb * nloc
    A = ATOT // NCORES
    g2f = g2.reshape(ATOT, nnei, din).astype(np.float32)
    h2f = h2.reshape(ATOT, nnei, 3).astype(np.float32)
    maskf = nlist_mask.reshape(ATOT, nnei)
    swf = sw.reshape(ATOT, nnei).astype(np.float32)

    g2s = g2f * swf[:, :, None]
    g2sTf = np.ascontiguousarray(g2s.transpose(0, 2, 1)).astype(np.float16)
    msw3 = (swf * maskf) / np.sqrt(np.float32(3.0))
    hcatf = np.concatenate([
        h2f.transpose(0, 2, 1),
        (h2f * msw3[:, :, None]).transpose(0, 2, 1),
    ], axis=1).astype(np.float16)
    hcatf = np.ascontiguousarray(hcatf)

    # W2 per head: Wqk columns c = d*8 + h; q heads h<4, k heads h>=4
    Wqk64 = Wqk.astype(np.float64).reshape(din, ND, 2 * NH)
    w2p = np.zeros((din, NH * ND), np.float16)
    for h in range(NH):
        Wq = Wqk64[:, :, h]          # [64, 64]
        Wk = Wqk64[:, :, NH + h]
        W2 = (Wq @ Wk.T) / np.sqrt(np.float64(ND))
        w2p[:, h * ND:(h + 1) * ND] = W2.astype(np.float16)

    rm = swf * maskf
    in_maps = []
    for c in range(NCORES):
        s = slice(c * A, (c + 1) * A)
        sws = np.concatenate([(SHIFT * swf[s]).T, rm[s].T], axis=1)
        in_maps.append({
            "g2sT": g2sTf[s],
            "hcat": hcatf[s],
            "w2p": w2p,
            "sws": np.ascontiguousarray(sws),
            "swrow": np.ascontiguousarray(swf[s].reshape(1, A * P)),
        })
    return in_maps, A


_NC_CACHE = {}


def kernel(g2, h2, nlist_mask, sw, Wqk, _trace=False, _trace_kwargs=None):
    nb, nloc, nnei, din = g2.shape
    in_maps, A = _host_prep(g2, h2, nlist_mask, sw, Wqk)
    key = A
    if key not in _NC_CACHE:
        _NC_CACHE[key] = build_nc(A)
    nc = _NC_CACHE[key]
    kw = {}
    if _trace:
        kw = dict(trace=True, **(_trace_kwargs or {}))
    res = run_bass_kernel_spmd(nc, in_maps, list(range(NCORES)), **kw)
    outs = [np.asarray(res.results[c]["out"]) for c in range(NCORES)]
    full = np.concatenate(outs, axis=0)  # [1024, 128, 4*128] bf16
    full = full.astype(np.float32).reshape(nb * nloc, nnei, NH, nnei)
    out = np.ascontiguousarray(full.transpose(0, 1, 3, 2)).reshape(
        nb, nloc, nnei, nnei, NH)
    if _trace:
        return out, res
    return out


if __name__ == "__main__":
    import reference as R
    inputs = {k: np.asarray(v) for k, v in R.setup_inputs().items()}
    out = kernel(**inputs)
    import jax.numpy as jnp
    ref = np.asarray(R.reference(**{k: jnp.asarray(v) for k, v in inputs.items()}))
    err = np.abs(out - ref)
    scale = np.abs(ref).max()
    print("absmax err:", err.max(), "scale:", scale, "scale-rel:", err.max() / scale)
    print("rel L2:", np.linalg.norm(err) / np.linalg.norm(ref))


# revision 19
# speedup vs baseline: 1.2842x; 1.2842x over previous
"""Trainium2 Bass kernel for nn_Atten2Map (DeePMD dpa2 Atten2Map-style sparse attention).

Contract: kernel(**inputs) takes FULL unsharded numpy inputs
(g2 [2,512,128,64], h2 [2,512,128,3], nlist_mask [2,512,128] bool,
sw [2,512,128], Wqk [64,512]) and returns the full output
[2,512,128,128,4] float32. Internally shards the nb*nloc=1024 atoms
data-parallel across 8 NeuronCores.

Math per atom (nnei=128 neighbors, ND=64, NH=4 heads):
  qk   = g2 @ Wqk                  -> q_h, k_h     [128, 64] each
  raw  = q_h @ k_h^T / sqrt(64)    (scores)
  hh   = h2 @ h2^T                 (gate)
  t    = (raw * hh + 20) * sw_i * sw_j - 20
  a    = softmax(t, axis=-1)  (constant shifts cancel in softmax)
  out[i, j, h] = a * mask_i * mask_j * sw_i * sw_j * hh / sqrt(3)

Device formulation (v2 — engine-balanced):
  host: g2s = g2 * sw            (so scores come out of PE already * sw_i*sw_j)
        W2_h = Wq_h @ Wk_h^T / 8 (64x64 per head; scores = G' W2 G'^T)
  PE:   ptm  = w2p2_hp^T @ G'^T           2-head-merged tmp  [128, 256] x2
        X'_h = tmpT_h^T @ G'^T            scores*sw_i*sw_j   [128, 128] x4/atom
        hh|hm = ht^T @ [ht|htm]           gates              [128, 256]/atom
  GPSIMD: v1 = X' (*) hh                  (PSUM read, gate multiply)
  ACT:  E1 = exp(v1 - 60)  -> bf16
        F  = exp(20*sw_i * sw_j) -> bf16  (scale=per-partition 20*sw_i)
  DVE:  E = E1 (*) F (bf16, 4x mode);  rows = reduce_X(E) f32
        rinva = (1/rows) * mask_i*sw_i
        ti_h = (E_h * rinva_h) * hm       (bf16 4x, per-head planes)
  out DRAM [A, 128(i), 4(h), 128(j)] bf16; host -> f32 transpose/reshape.
"""

import numpy as np
from contextlib import ExitStack

import ml_dtypes
import concourse.bass as bass
import concourse.tile as tile
from concourse import bacc, mybir
from concourse.bass_utils import run_bass_kernel_spmd

ND, NH, SHIFT = 64, 4, 20.0
NNEI, DIN = 128, 64
NCORES = 8
EXPB = 60.0  # constant shift inside exp; cancels in softmax normalization

F32 = mybir.dt.float32
F16 = mybir.dt.float16
BF16 = mybir.dt.bfloat16
DTE = mybir.dt.bfloat16  # post-exp path dtype

P = NNEI     # 128
BLK = 16     # atoms per DMA block
GRP = 4      # atoms per output DMA group


def build_nc(A: int):
    """Build the per-core Bass program for A atoms (A multiple of BLK)."""
    assert A % BLK == 0
    nc = bacc.Bacc("TRN2", target_bir_lowering=False, debug=False, num_devices=NCORES)
    dp = nc.declare_dram_parameter
    g2sT = dp("g2sT", [A, DIN, P], F16, isOutput=False)    # sw-folded g2, transposed
    hcat = dp("hcat", [A, 6, P], F16, isOutput=False)      # rows 0:3 h2T, 3:6 (h2*mask*sw/sqrt3)T
    w2p = dp("w2p", [DIN, NH * ND], F16, isOutput=False)   # per-head W2, head-major cols
    sws = dp("sws", [P, 2 * A], F32, isOutput=False)       # [20*swiT | rmT]
    swrow = dp("swrow", [1, A * P], F16, isOutput=False)   # flat sw rows for broadcast
    out = dp("out", [A, P, NH * P], DTE, isOutput=True)   # [atom, i, (h, j)]

    AF = mybir.ActivationFunctionType
    OP = mybir.AluOpType

    with tile.TileContext(nc) as tc, ExitStack() as ctx:
        sb = ctx.enter_context(tc.tile_pool(name="persist", bufs=1))
        w2p_s = sb.tile([DIN, NH * ND], F16)
        nc.sync.dma_start(w2p_s[:, :], w2p[:, :])
        sws_s = sb.tile([P, 2 * A], F32)
        nc.sync.dma_start(sws_s[:, :], sws[:, :])
        swi20T_s = sws_s[:, 0:A]
        rmT_s = sws_s[:, A:2 * A]
        negb = sb.tile([P, 1], F32)
        nc.vector.memset(negb[:, :], -EXPB)

        # block-level input pools
        gt_pool = ctx.enter_context(tc.tile_pool(name="gt", bufs=2))
        hc_pool = ctx.enter_context(tc.tile_pool(name="hc", bufs=2))
        swj_pool = ctx.enter_context(tc.tile_pool(name="swj", bufs=2))
        # pair/atom-level pools
        tts_pool = ctx.enter_context(tc.tile_pool(name="tts", bufs=3))
        hh_pool = ctx.enter_context(tc.tile_pool(name="hh", bufs=3))
        v1_pool = ctx.enter_context(tc.tile_pool(name="v1", bufs=3))
        w20_pool = ctx.enter_context(tc.tile_pool(name="w20", bufs=3))
        v2_pool = ctx.enter_context(tc.tile_pool(name="v2", bufs=3))
        e_pool = ctx.enter_context(tc.tile_pool(name="e", bufs=3))
        e2_pool = ctx.enter_context(tc.tile_pool(name="e2", bufs=3))
        stat_pool = ctx.enter_context(tc.tile_pool(name="stat", bufs=4))
        stage_pool = ctx.enter_context(tc.tile_pool(name="stage", bufs=2))
        # PSUM pools
        ptm_pool = ctx.enter_context(tc.tile_pool(name="ptm", bufs=2, space="PSUM"))
        psc_pool = ctx.enter_context(tc.tile_pool(name="psc", bufs=2, space="PSUM"))
        phh_pool = ctx.enter_context(tc.tile_pool(name="phh", bufs=2, space="PSUM"))

        for blk in range(A // BLK):
            b0 = blk * BLK
            gts = gt_pool.tile([DIN, BLK * P], F16)
            nc.sync.dma_start(gts[:, :].rearrange("p (a j) -> p a j", a=BLK),
                              g2sT[b0:b0 + BLK, :, :].rearrange("a p j -> p a j"))
            hcs = hc_pool.tile([3, BLK * P], F16, tag="ht")
            nc.sync.dma_start(hcs[:, :].rearrange("p (a j) -> p a j", a=BLK),
                              hcat[b0:b0 + BLK, 0:3, :].rearrange("a p j -> p a j"))
            hms = hc_pool.tile([3, BLK * P], F16, tag="htm")
            nc.sync.dma_start(hms[:, :].rearrange("p (a j) -> p a j", a=BLK),
                              hcat[b0:b0 + BLK, 3:6, :].rearrange("a p j -> p a j"))
            swjb = swj_pool.tile([P, BLK * P], F16)
            nc.sync.dma_start(swjb[:, :],
                              swrow[0:1, b0 * P:(b0 + BLK) * P].broadcast_to([P, BLK * P]))

            stage = None
            for lp in range(BLK // 2):
                la0 = 2 * lp          # block-local atom indices
                a0 = b0 + la0         # global (per-core) atom index
                gpair = gts[:, la0 * P:(la0 + 2) * P]          # [64, 256]

                # --- tmp matmuls: 2-head-merged, per pair ---
                ptm = ptm_pool.tile([P, 2 * 2 * P], F32)
                for hp in range(2):
                    nc.tensor.matmul(ptm[:, hp * 2 * P:(hp + 1) * 2 * P],
                                     w2p_s[:, hp * 2 * ND:(hp + 1) * 2 * ND],
                                     gpair, start=True, stop=True)
                tts = tts_pool.tile([P, 2 * 2 * P], F16, tag="tts")
                nc.scalar.copy(tts[:, :], ptm[:, :])
                tts2 = tts_pool.tile([ND, 2 * 2 * P], F16, tag="tts2")
                nc.sync.dma_start(tts2[:, :], tts[ND:, :])

                # --- gate matmuls: hh|hm per atom, [128, 512] per pair ---
                phh = phh_pool.tile([P, 4 * P], F32)
                for ai in range(2):
                    hta = hcs[:, (la0 + ai) * P:(la0 + ai + 1) * P]
                    htma = hms[:, (la0 + ai) * P:(la0 + ai + 1) * P]
                    nc.tensor.matmul(phh[:, (2 * ai) * P:(2 * ai + 1) * P],
                                     hta, hta, start=True, stop=True)
                    nc.tensor.matmul(phh[:, (2 * ai + 1) * P:(2 * ai + 2) * P],
                                     hta, htma, start=True, stop=True)
                hhm = hh_pool.tile([P, 4 * P], F16, tag="hhm")
                nc.scalar.copy(hhm[:, :], phh[:, :])

                # --- score matmuls: per atom per head, N=128 ---
                psc = psc_pool.tile([P, 2 * NH * P], F32)
                for ai in range(2):
                    for hp in range(2):
                        for hi in range(2):
                            h = 2 * hp + hi
                            src_t = tts if hi == 0 else tts2
                            lhsT = src_t[0:ND,
                                         hp * 2 * P + ai * P:hp * 2 * P + (ai + 1) * P]
                            ga = gts[:, (la0 + ai) * P:(la0 + ai + 1) * P]
                            nc.tensor.matmul(psc[:, (ai * NH + h) * P:(ai * NH + h + 1) * P],
                                             lhsT, ga, start=True, stop=True)

                # --- v1 = X' * hh   (DVE, PSUM read; gpsimd cannot touch PSUM) ---
                v1 = v1_pool.tile([P, 2 * NH * P], F16, tag="v1")
                hh_b = hhm[:, :].rearrange("p (a k j) -> p a k j", a=2, k=2)[:, :, 0:1, :] \
                    .broadcast_to([P, 2, NH, P])
                nc.vector.tensor_tensor(
                    v1[:, :].rearrange("p (a h j) -> p a h j", a=2, h=NH),
                    psc[:, :].rearrange("p (a h j) -> p a h j", a=2, h=NH),
                    hh_b, op=OP.mult)

                # --- w20 = 20*swi * swj (scalar ACT copy, exact f32) ---
                w20 = w20_pool.tile([P, 2 * P], F16, tag="w20")
                for ai in range(2):
                    nc.scalar.activation(w20[:, ai * P:(ai + 1) * P],
                                         swjb[:, (la0 + ai) * P:(la0 + ai + 1) * P],
                                         AF.Copy,
                                         scale=swi20T_s[:, a0 + ai:a0 + ai + 1])

                # --- v2 = v1 + w20 (gpsimd, all-SBUF) ---
                v2 = v2_pool.tile([P, 2 * NH * P], F16, tag="v2")
                w20_b = w20[:, :].rearrange("p (a j) -> p a j", a=2) \
                    .unsqueeze(2).broadcast_to([P, 2, NH, P])
                nc.gpsimd.tensor_tensor(
                    v2[:, :].rearrange("p (a h j) -> p a h j", a=2, h=NH),
                    v1[:, :].rearrange("p (a h j) -> p a h j", a=2, h=NH),
                    w20_b, op=OP.add)

                # --- E = exp(v2 - 60) -> bf16 (pair-wide) ---
                e_t = e_pool.tile([P, 2 * NH * P], DTE, tag="e")
                nc.scalar.activation(e_t[:, :], v2[:, :], AF.Exp,
                                     bias=negb[:, 0:1], scale=1.0)

                # --- rows = sum_j E; rinva = mask_i*swi / rows (bf16) ---
                rows = stat_pool.tile([P, 4 * NH], F32, tag="rows")
                nc.vector.tensor_reduce(
                    rows[:, 0:2 * NH],
                    e_t[:, :].rearrange("p (k j) -> p k j", k=2 * NH),
                    axis=mybir.AxisListType.X, op=OP.add)
                nc.vector.reciprocal(rows[:, 2 * NH:4 * NH], rows[:, 0:2 * NH])
                rinva = stat_pool.tile([P, 2 * NH], DTE, tag="rinva")
                for ai in range(2):
                    nc.vector.tensor_scalar(
                        rinva[:, ai * NH:(ai + 1) * NH],
                        rows[:, 2 * NH + ai * NH:2 * NH + (ai + 1) * NH],
                        rmT_s[:, a0 + ai:a0 + ai + 1], None, op0=OP.mult)

                # --- out: ti = E * rinva * hm as two pair-wide bf16 4x TTs ---
                gi = (la0 // GRP)
                if la0 % GRP == 0:
                    stage = stage_pool.tile([P, GRP * NH * P], DTE, tag="stage")
                soff = (la0 % GRP) * NH * P
                e2 = e2_pool.tile([P, 2 * NH * P], DTE, tag="e2")
                rinva_b = rinva[:, :].rearrange("p (a h) -> p a h", a=2) \
                    .unsqueeze(3).broadcast_to([P, 2, NH, P])
                nc.vector.tensor_tensor(
                    e2[:, :].rearrange("p (a h j) -> p a h j", a=2, h=NH),
                    e_t[:, :].rearrange("p (a h j) -> p a h j", a=2, h=NH),
                    rinva_b, op=OP.mult)
                hm_b = hhm[:, :].rearrange("p (a k j) -> p a k j", a=2, k=2)[:, :, 1:2, :] \
                    .broadcast_to([P, 2, NH, P])
                nc.gpsimd.tensor_tensor(
                    stage[:, soff:soff + 2 * NH * P]
                        .rearrange("p (a h j) -> p a h j", a=2, h=NH),
                    e2[:, :].rearrange("p (a h j) -> p a h j", a=2, h=NH),
                    hm_b, op=OP.mult)
                if la0 % GRP == GRP - 2:
                    g0 = b0 + gi * GRP
                    nc.sync.dma_start(
                        out[g0:g0 + GRP, :, :].rearrange("a p q -> p a q"),
                        stage[:, :].rearrange("p (a q) -> p a q", a=GRP))
    if not nc.is_finalized():
        nc.finalize()
    return nc


def _host_prep(g2, h2, nlist_mask, sw, Wqk):
    """Build per-core input maps (host-side numpy prep)."""
    nb, nloc, nnei, din = g2.shape
    ATOT = nb * nloc
    A = ATOT // NCORES
    g2f = g2.reshape(ATOT, nnei, din).astype(np.float32)
    h2f = h2.reshape(ATOT, nnei, 3).astype(np.float32)
    maskf = nlist_mask.reshape(ATOT, nnei)
    swf = sw.reshape(ATOT, nnei).astype(np.float32)

    g2s = g2f * swf[:, :, None]
    g2sTf = np.ascontiguousarray(g2s.transpose(0, 2, 1)).astype(np.float16)
    msw3 = (swf * maskf) / np.sqrt(np.float32(3.0))
    hcatf = np.concatenate([
        h2f.transpose(0, 2, 1),
        (h2f * msw3[:, :, None]).transpose(0, 2, 1),
    ], axis=1).astype(np.float16)
    hcatf = np.ascontiguousarray(hcatf)

    # W2 per head: Wqk columns c = d*8 + h; q heads h<4, k heads h>=4
    Wqk64 = Wqk.astype(np.float64).reshape(din, ND, 2 * NH)
    w2p = np.zeros((din, NH * ND), np.float16)
    for h in range(NH):
        Wq = Wqk64[:, :, h]          # [64, 64]
        Wk = Wqk64[:, :, NH + h]
        W2 = (Wq @ Wk.T) / np.sqrt(np.float64(ND))
        w2p[:, h * ND:(h + 1) * ND] = W2.astype(np.float16)

    rm = swf * maskf
    in_maps = []
    for c in range(NCORES):
        s = slice(c * A, (c + 1) * A)
        sws = np.concatenate([(SHIFT * swf[s]).T, rm[s].T], axis=1)
        in_maps.append({
            "g2sT": g2sTf[s],
            "hcat": hcatf[s],
            "w2p": w2p,
            "sws": np.ascontiguousarray(sws),
            "swrow": np.ascontiguousarray(swf[s].reshape(1, A * P)).astype(np.float16),
        })
    return in_maps, A


_NC_CACHE = {}


def kernel(g2, h2, nlist_mask, sw, Wqk, _trace=False, _trace_kwargs=None):
    nb, nloc, nnei, din = g2.shape
    in_maps, A = _host_prep(g2, h2, nlist_mask, sw, Wqk)
    key = A
    if key not in _NC_CACHE:
        _NC_CACHE[key] = build_nc(A)
    nc = _NC_CACHE[key]
    kw = {}
    if _trace:
        kw = dict(trace=True, **(_trace_kwargs or {}))
    res = run_bass_kernel_spmd(nc, in_maps, list(range(NCORES)), **kw)
    outs = [np.asarray(res.results[c]["out"]) for c in range(NCORES)]
    full = np.concatenate(outs, axis=0)  # [1024, 128, 4*128] bf16
    full = full.astype(np.float32).reshape(nb * nloc, nnei, NH, nnei)
    out = np.ascontiguousarray(full.transpose(0, 1, 3, 2)).reshape(
        nb, nloc, nnei, nnei, NH)
    if _trace:
        return out, res
    return out


if __name__ == "__main__":
    import reference as R
    inputs = {k: np.asarray(v) for k, v in R.setup_inputs().items()}
    out = kernel(**inputs)
    import jax.numpy as jnp
    ref = np.asarray(R.reference(**{k: jnp.asarray(v) for k, v in inputs.items()}))
    err = np.abs(out - ref)
    scale = np.abs(ref).max()
    print("absmax err:", err.max(), "scale:", scale, "scale-rel:", err.max() / scale)
    print("rel L2:", np.linalg.norm(err) / np.linalg.norm(ref))


# revision 21
# speedup vs baseline: 1.5770x; 1.2280x over previous
"""Trainium2 Bass kernel for nn_Atten2Map (DeePMD dpa2 Atten2Map-style sparse attention).

Contract: kernel(**inputs) takes FULL unsharded numpy inputs
(g2 [2,512,128,64], h2 [2,512,128,3], nlist_mask [2,512,128] bool,
sw [2,512,128], Wqk [64,512]) and returns the full output
[2,512,128,128,4] float32. Internally shards the nb*nloc=1024 atoms
data-parallel across 8 NeuronCores.

Math per atom (nnei=128 neighbors, ND=64, NH=4 heads):
  qk   = g2 @ Wqk                  -> q_h, k_h     [128, 64] each
  raw  = q_h @ k_h^T / sqrt(64)    (scores)
  hh   = h2 @ h2^T                 (gate)
  t    = (raw * hh + 20) * sw_i * sw_j - 20
  a    = softmax(t, axis=-1)  (constant shifts cancel in softmax)
  out[i, j, h] = a * mask_i * mask_j * sw_i * sw_j * hh / sqrt(3)

Device formulation (v2 — engine-balanced):
  host: g2s = g2 * sw            (so scores come out of PE already * sw_i*sw_j)
        W2_h = Wq_h @ Wk_h^T / 8 (64x64 per head; scores = G' W2 G'^T)
  PE:   ptm  = w2p2_hp^T @ G'^T           2-head-merged tmp  [128, 256] x2
        X'_h = tmpT_h^T @ G'^T            scores*sw_i*sw_j   [128, 128] x4/atom
        hh|hm = ht^T @ [ht|htm]           gates              [128, 256]/atom
  GPSIMD: v1 = X' (*) hh                  (PSUM read, gate multiply)
  ACT:  E1 = exp(v1 - 60)  -> bf16
        F  = exp(20*sw_i * sw_j) -> bf16  (scale=per-partition 20*sw_i)
  DVE:  E = E1 (*) F (bf16, 4x mode);  rows = reduce_X(E) f32
        rinva = (1/rows) * mask_i*sw_i
        ti_h = (E_h * rinva_h) * hm       (bf16 4x, per-head planes)
  out DRAM [A, 128(i), 4(h), 128(j)] bf16; host -> f32 transpose/reshape.
"""

import numpy as np
from contextlib import ExitStack

import ml_dtypes
import concourse.bass as bass
import concourse.tile as tile
from concourse import bacc, mybir
from concourse.bass_utils import run_bass_kernel_spmd

ND, NH, SHIFT = 64, 4, 20.0
NNEI, DIN = 128, 64
NCORES = 8
EXPB = 60.0  # constant shift inside exp; cancels in softmax normalization

F32 = mybir.dt.float32
F16 = mybir.dt.float16
BF16 = mybir.dt.bfloat16
DTE = mybir.dt.bfloat16  # post-exp path dtype

P = NNEI     # 128
BLK = 16     # atoms per DMA block
GRP = 4      # atoms per output DMA group


def build_nc(A: int):
    """Build the per-core Bass program for A atoms (A multiple of BLK)."""
    assert A % BLK == 0
    nc = bacc.Bacc("TRN2", target_bir_lowering=False, debug=False, num_devices=NCORES)
    dp = nc.declare_dram_parameter
    g2sT = dp("g2sT", [A, DIN, P], F16, isOutput=False)    # sw-folded g2, transposed
    hcat = dp("hcat", [A, 6, P], F16, isOutput=False)      # rows 0:3 h2T, 3:6 (h2*mask*sw/sqrt3)T
    w2p = dp("w2p", [DIN, NH * ND], F16, isOutput=False)   # per-head W2, head-major cols
    sws = dp("sws", [P, 2 * A], F32, isOutput=False)       # [20*swiT | rmT]
    swrow = dp("swrow", [1, A * P], F16, isOutput=False)   # flat sw rows for broadcast
    out = dp("out", [A, P, NH * P], DTE, isOutput=True)   # [atom, i, (h, j)]

    AF = mybir.ActivationFunctionType
    OP = mybir.AluOpType

    with tile.TileContext(nc) as tc, ExitStack() as ctx:
        sb = ctx.enter_context(tc.tile_pool(name="persist", bufs=1))
        w2p_s = sb.tile([DIN, NH * ND], F16)
        nc.sync.dma_start(w2p_s[:, :], w2p[:, :])
        sws_s = sb.tile([P, 2 * A], F32)
        nc.sync.dma_start(sws_s[:, :], sws[:, :])
        swi20T_s = sws_s[:, 0:A]
        rmT_s = sws_s[:, A:2 * A]
        negb = sb.tile([P, 1], F32)
        nc.vector.memset(negb[:, :], -EXPB)

        # block-level input pools
        gt_pool = ctx.enter_context(tc.tile_pool(name="gt", bufs=2))
        hc_pool = ctx.enter_context(tc.tile_pool(name="hc", bufs=2))
        swj_pool = ctx.enter_context(tc.tile_pool(name="swj", bufs=2))
        # pair/atom-level pools
        tts_pool = ctx.enter_context(tc.tile_pool(name="tts", bufs=4))
        hh_pool = ctx.enter_context(tc.tile_pool(name="hh", bufs=4))
        v1_pool = ctx.enter_context(tc.tile_pool(name="v1", bufs=4))
        w20_pool = ctx.enter_context(tc.tile_pool(name="w20", bufs=4))
        v2_pool = ctx.enter_context(tc.tile_pool(name="v2", bufs=4))
        e_pool = ctx.enter_context(tc.tile_pool(name="e", bufs=4))
        e2_pool = ctx.enter_context(tc.tile_pool(name="e2", bufs=4))
        stat_pool = ctx.enter_context(tc.tile_pool(name="stat", bufs=8))
        stage_pool = ctx.enter_context(tc.tile_pool(name="stage", bufs=2))
        # PSUM pools
        ptm_pool = ctx.enter_context(tc.tile_pool(name="ptm", bufs=2, space="PSUM"))
        psc_pool = ctx.enter_context(tc.tile_pool(name="psc", bufs=2, space="PSUM"))
        phh_pool = ctx.enter_context(tc.tile_pool(name="phh", bufs=2, space="PSUM"))

        for blk in range(A // BLK):
            b0 = blk * BLK
            gts = gt_pool.tile([DIN, BLK * P], F16)
            nc.sync.dma_start(gts[:, :].rearrange("p (a j) -> p a j", a=BLK),
                              g2sT[b0:b0 + BLK, :, :].rearrange("a p j -> p a j"))
            hcs = hc_pool.tile([3, BLK * P], F16, tag="ht")
            nc.sync.dma_start(hcs[:, :].rearrange("p (a j) -> p a j", a=BLK),
                              hcat[b0:b0 + BLK, 0:3, :].rearrange("a p j -> p a j"))
            hms = hc_pool.tile([3, BLK * P], F16, tag="htm")
            nc.sync.dma_start(hms[:, :].rearrange("p (a j) -> p a j", a=BLK),
                              hcat[b0:b0 + BLK, 3:6, :].rearrange("a p j -> p a j"))
            swjb = swj_pool.tile([P, BLK * P], F16)
            nc.sync.dma_start(swjb[:, :],
                              swrow[0:1, b0 * P:(b0 + BLK) * P].broadcast_to([P, BLK * P]))

            stage = None
            for lp in range(BLK // 2):
                la0 = 2 * lp          # block-local atom indices
                a0 = b0 + la0         # global (per-core) atom index
                gpair = gts[:, la0 * P:(la0 + 2) * P]          # [64, 256]

                # --- tmp matmuls: 2-head-merged, per pair ---
                ptm = ptm_pool.tile([P, 2 * 2 * P], F32)
                for hp in range(2):
                    nc.tensor.matmul(ptm[:, hp * 2 * P:(hp + 1) * 2 * P],
                                     w2p_s[:, hp * 2 * ND:(hp + 1) * 2 * ND],
                                     gpair, start=True, stop=True)
                tts = tts_pool.tile([P, 2 * 2 * P], F16, tag="tts")
                nc.scalar.copy(tts[:, :], ptm[:, :])
                tts2 = tts_pool.tile([ND, 2 * 2 * P], F16, tag="tts2")
                nc.sync.dma_start(tts2[:, :], tts[ND:, :])

                # --- gate matmuls: hh|hm per atom, [128, 512] per pair ---
                phh = phh_pool.tile([P, 4 * P], F32)
                for ai in range(2):
                    hta = hcs[:, (la0 + ai) * P:(la0 + ai + 1) * P]
                    htma = hms[:, (la0 + ai) * P:(la0 + ai + 1) * P]
                    nc.tensor.matmul(phh[:, (2 * ai) * P:(2 * ai + 1) * P],
                                     hta, hta, start=True, stop=True)
                    nc.tensor.matmul(phh[:, (2 * ai + 1) * P:(2 * ai + 2) * P],
                                     hta, htma, start=True, stop=True)
                hhm = hh_pool.tile([P, 4 * P], F16, tag="hhm")
                nc.scalar.copy(hhm[:, :], phh[:, :])

                # --- score matmuls: per atom per head, N=128 ---
                psc = psc_pool.tile([P, 2 * NH * P], F32)
                for ai in range(2):
                    for hp in range(2):
                        for hi in range(2):
                            h = 2 * hp + hi
                            src_t = tts if hi == 0 else tts2
                            lhsT = src_t[0:ND,
                                         hp * 2 * P + ai * P:hp * 2 * P + (ai + 1) * P]
                            ga = gts[:, (la0 + ai) * P:(la0 + ai + 1) * P]
                            nc.tensor.matmul(psc[:, (ai * NH + h) * P:(ai * NH + h + 1) * P],
                                             lhsT, ga, start=True, stop=True)

                # --- v1 = X' * hh   (DVE, PSUM read; gpsimd cannot touch PSUM) ---
                v1 = v1_pool.tile([P, 2 * NH * P], F16, tag="v1")
                hh_b = hhm[:, :].rearrange("p (a k j) -> p a k j", a=2, k=2)[:, :, 0:1, :] \
                    .broadcast_to([P, 2, NH, P])
                nc.vector.tensor_tensor(
                    v1[:, :].rearrange("p (a h j) -> p a h j", a=2, h=NH),
                    psc[:, :].rearrange("p (a h j) -> p a h j", a=2, h=NH),
                    hh_b, op=OP.mult)

                # --- w20 = 20*swi * swj (scalar ACT copy, exact f32) ---
                w20 = w20_pool.tile([P, 2 * P], F16, tag="w20")
                for ai in range(2):
                    nc.scalar.activation(w20[:, ai * P:(ai + 1) * P],
                                         swjb[:, (la0 + ai) * P:(la0 + ai + 1) * P],
                                         AF.Copy,
                                         scale=swi20T_s[:, a0 + ai:a0 + ai + 1])

                # --- v2 = v1 + w20 (gpsimd, all-SBUF) ---
                v2 = v2_pool.tile([P, 2 * NH * P], F16, tag="v2")
                w20_b = w20[:, :].rearrange("p (a j) -> p a j", a=2) \
                    .unsqueeze(2).broadcast_to([P, 2, NH, P])
                nc.gpsimd.tensor_tensor(
                    v2[:, :].rearrange("p (a h j) -> p a h j", a=2, h=NH),
                    v1[:, :].rearrange("p (a h j) -> p a h j", a=2, h=NH),
                    w20_b, op=OP.add)

                # --- E = exp(v2 - 60) -> bf16 (pair-wide) ---
                e_t = e_pool.tile([P, 2 * NH * P], DTE, tag="e")
                nc.scalar.activation(e_t[:, :], v2[:, :], AF.Exp,
                                     bias=negb[:, 0:1], scale=1.0)

                # --- rows = sum_j E; rinva = mask_i*swi / rows (bf16) ---
                rows = stat_pool.tile([P, 4 * NH], F32, tag="rows")
                nc.vector.tensor_reduce(
                    rows[:, 0:2 * NH],
                    e_t[:, :].rearrange("p (k j) -> p k j", k=2 * NH),
                    axis=mybir.AxisListType.X, op=OP.add)
                nc.vector.reciprocal(rows[:, 2 * NH:4 * NH], rows[:, 0:2 * NH])
                rinva = stat_pool.tile([P, 2 * NH], DTE, tag="rinva")
                for ai in range(2):
                    nc.vector.tensor_scalar(
                        rinva[:, ai * NH:(ai + 1) * NH],
                        rows[:, 2 * NH + ai * NH:2 * NH + (ai + 1) * NH],
                        rmT_s[:, a0 + ai:a0 + ai + 1], None, op0=OP.mult)

                # --- out: ti = E * rinva * hm as two pair-wide bf16 4x TTs ---
                gi = (la0 // GRP)
                if la0 % GRP == 0:
                    stage = stage_pool.tile([P, GRP * NH * P], DTE, tag="stage")
                soff = (la0 % GRP) * NH * P
                zt = e2_pool.tile([P, 2 * NH * P], BF16, tag="z")
                rinva_b = rinva[:, :].rearrange("p (a h) -> p a h", a=2) \
                    .unsqueeze(3).broadcast_to([P, 2, NH, P])
                hm_b = hhm[:, :].rearrange("p (a k j) -> p a k j", a=2, k=2)[:, :, 1:2, :] \
                    .broadcast_to([P, 2, NH, P])
                nc.gpsimd.tensor_tensor(
                    zt[:, :].rearrange("p (a h j) -> p a h j", a=2, h=NH),
                    rinva_b, hm_b, op=OP.mult)
                nc.vector.tensor_tensor(
                    stage[:, soff:soff + 2 * NH * P],
                    e_t[:, :], zt[:, :], op=OP.mult)
                if la0 % GRP == GRP - 2:
                    g0 = b0 + gi * GRP
                    nc.sync.dma_start(
                        out[g0:g0 + GRP, :, :].rearrange("a p q -> p a q"),
                        stage[:, :].rearrange("p (a q) -> p a q", a=GRP))
    if not nc.is_finalized():
        nc.finalize()
    return nc


def _host_prep(g2, h2, nlist_mask, sw, Wqk):
    """Build per-core input maps (host-side numpy prep)."""
    nb, nloc, nnei, din = g2.shape
    ATOT = nb * nloc
    A = ATOT // NCORES
    g2f = g2.reshape(ATOT, nnei, din).astype(np.float32)
    h2f = h2.reshape(ATOT, nnei, 3).astype(np.float32)
    maskf = nlist_mask.reshape(ATOT, nnei)
    swf = sw.reshape(ATOT, nnei).astype(np.float32)

    g2s = g2f * swf[:, :, None]
    g2sTf = np.ascontiguousarray(g2s.transpose(0, 2, 1)).astype(np.float16)
    msw3 = (swf * maskf) / np.sqrt(np.float32(3.0))
    hcatf = np.concatenate([
        h2f.transpose(0, 2, 1),
        (h2f * msw3[:, :, None]).transpose(0, 2, 1),
    ], axis=1).astype(np.float16)
    hcatf = np.ascontiguousarray(hcatf)

    # W2 per head: Wqk columns c = d*8 + h; q heads h<4, k heads h>=4
    Wqk64 = Wqk.astype(np.float64).reshape(din, ND, 2 * NH)
    w2p = np.zeros((din, NH * ND), np.float16)
    for h in range(NH):
        Wq = Wqk64[:, :, h]          # [64, 64]
        Wk = Wqk64[:, :, NH + h]
        W2 = (Wq @ Wk.T) / np.sqrt(np.float64(ND))
        w2p[:, h * ND:(h + 1) * ND] = W2.astype(np.float16)

    rm = swf * maskf
    in_maps = []
    for c in range(NCORES):
        s = slice(c * A, (c + 1) * A)
        sws = np.concatenate([(SHIFT * swf[s]).T, rm[s].T], axis=1)
        in_maps.append({
            "g2sT": g2sTf[s],
            "hcat": hcatf[s],
            "w2p": w2p,
            "sws": np.ascontiguousarray(sws),
            "swrow": np.ascontiguousarray(swf[s].reshape(1, A * P)).astype(np.float16),
        })
    return in_maps, A


_NC_CACHE = {}


def kernel(g2, h2, nlist_mask, sw, Wqk, _trace=False, _trace_kwargs=None):
    nb, nloc, nnei, din = g2.shape
    in_maps, A = _host_prep(g2, h2, nlist_mask, sw, Wqk)
    key = A
    if key not in _NC_CACHE:
        _NC_CACHE[key] = build_nc(A)
    nc = _NC_CACHE[key]
    kw = {}
    if _trace:
        kw = dict(trace=True, **(_trace_kwargs or {}))
    res = run_bass_kernel_spmd(nc, in_maps, list(range(NCORES)), **kw)
    outs = [np.asarray(res.results[c]["out"]) for c in range(NCORES)]
    full = np.concatenate(outs, axis=0)  # [1024, 128, 4*128] bf16
    full = full.astype(np.float32).reshape(nb * nloc, nnei, NH, nnei)
    out = np.ascontiguousarray(full.transpose(0, 1, 3, 2)).reshape(
        nb, nloc, nnei, nnei, NH)
    if _trace:
        return out, res
    return out


if __name__ == "__main__":
    import reference as R
    inputs = {k: np.asarray(v) for k, v in R.setup_inputs().items()}
    out = kernel(**inputs)
    import jax.numpy as jnp
    ref = np.asarray(R.reference(**{k: jnp.asarray(v) for k, v in inputs.items()}))
    err = np.abs(out - ref)
    scale = np.abs(ref).max()
    print("absmax err:", err.max(), "scale:", scale, "scale-rel:", err.max() / scale)
    print("rel L2:", np.linalg.norm(err) / np.linalg.norm(ref))
